# revision 37
# baseline (speedup 1.0000x reference)
"""Multi-head attention (B=8, N=1024, D=768, H=12, softmax over full dim-scaled
scores) on 8 Trainium2 NeuronCores, data-parallel over the batch dimension:
core b computes batch element b end-to-end; no collectives.

v2 schedule (from trace analysis of the 204 us baseline):
  - Host pre-packs every tensor into its exact SBUF layout so the input
    phase is 7 large DMAs spread over 4 queues (baseline: 43 small DMAs,
    ~8 us of serialized issue).
  - The attention middle phase is ScalarE-bound (96 exp ACTs ~1.11 us each).
    Scores PSUM is one rotating 2-deep pool consumed in strict A/B
    alternation so the exp stream never waits on a PSUM bank; PV groups and
    projection/output-projection filler are interleaved between the scores
    matmuls at sub-group granularity to keep PE warm without delaying them.
  - Softmax denominators: batched reciprocal_approx_fast (51 ULP, 5x faster
    than the exact DVE reciprocal that cost 3.3 us per pair).
  - Output projection pass-1 (bias + ct0..4 of the contraction) runs as PE
    filler inside the pair-5 loop; only the ct5 rank-128 update, the final
    add and the stores remain after the drain. Stores are 8 full-row DMAs
    alternating between two queues.
"""

import numpy as np
import ml_dtypes

import concourse.bass as bass
import concourse.bacc as bacc
import concourse.tile as tile
from concourse import mybir
from concourse.bass_utils import run_bass_kernel_spmd

f32 = mybir.dt.float32
bf16 = mybir.dt.bfloat16

B = 8
N = 1024
D = 768
H = 12
DH = 64
SCALE = float(D) ** -0.5
NT = N // 128   # 8 sequence tiles
KT = D // 128   # 6 feature tiles
NPAIR = H // 2  # 6 head pairs


def build_bass():
    nc = bacc.Bacc("TRN2", target_bir_lowering=False, debug=False, num_devices=B)
    # all inputs pre-packed on host to [partition, kt, col] SBUF layouts;
    # each DMA's SBUF destination is fully contiguous (strided destinations
    # fragment the transfer into ~1 KB packets and halve queue bandwidth)
    x_d = [nc.dram_tensor(f"x{kt}", [128, N], bf16, kind="ExternalInput")
           for kt in range(KT)]
    wqk0_d = nc.dram_tensor("wqk0", [128, KT, 256], bf16, kind="ExternalInput")
    wqkra_d = nc.dram_tensor("wqkra", [128, 3, 1280], bf16, kind="ExternalInput")
    wqkrb_d = nc.dram_tensor("wqkrb", [128, 3, 1280], bf16, kind="ExternalInput")
    wv_d = nc.dram_tensor("wv", [128, KT, D], bf16, kind="ExternalInput")
    wo_d = nc.dram_tensor("wo", [128, KT, D], bf16, kind="ExternalInput")
    bo_d = nc.dram_tensor("bo", [D], f32, kind="ExternalInput")
    out_d = nc.dram_tensor("out", [N, D], f32, kind="ExternalOutput")

    with tile.TileContext(nc) as tc:
        with tc.tile_pool(name="persist", bufs=1) as pp:
            # persistent SBUF tensors
            xT_sb = pp.tile([128, KT, N], bf16)        # x^T feature tiles
            wqk0_sb = pp.tile([128, KT, 256], bf16)    # pair-0 q|k cols
            wqkr_sb = pp.tile([128, KT, 1280], bf16)   # pair 1-5 q|k cols
            wv_sb = pp.tile([128, KT, D], bf16)        # v cols (head order)
            wo_sb = pp.tile([128, KT, D], bf16)        # W_out feature tiles
            qkT = pp.tile([128, 2 * KT, N], bf16)      # q,k feature-major
            vaug = pp.tile([128, NT, H, DH + 1], bf16)  # v token-major + ones
            aoT = pp.tile([128, KT, N], bf16)          # attention out, feature-major
            osb_all = pp.tile([128, NT, 2, 384], bf16)  # out-proj pass-1 partials
            bias_f32 = pp.tile([1, D], f32)
            bias_bf = pp.tile([1, D], bf16)
            ones_col = pp.tile([1, 128], bf16)

            # ---- input DMAs: contiguous destinations, three queues, ordered
            # by first use. x feature tiles split per kt so the first qk
            # projection chunk can accumulate in landing order instead of
            # waiting for the full 1.6 MB of x.
            nc.sync.dma_start(out=xT_sb[:, 0, :], in_=x_d[0][:, :])
            nc.scalar.dma_start(out=xT_sb[:, 1, :], in_=x_d[1][:, :])
            nc.gpsimd.dma_start(out=wqk0_sb, in_=wqk0_d[:, :, :])
            nc.sync.dma_start(out=xT_sb[:, 3, :], in_=x_d[3][:, :])
            nc.scalar.dma_start(out=xT_sb[:, 4, :], in_=x_d[4][:, :])
            nc.gpsimd.dma_start(out=xT_sb[:, 2, :], in_=x_d[2][:, :])
            nc.sync.dma_start(out=xT_sb[:, 5, :], in_=x_d[5][:, :])
            nc.gpsimd.dma_start(out=wv_sb[:, 3:6, :], in_=wv_d[:, 3:6, :])
            nc.sync.dma_start(out=wv_sb[:, 0:3, :], in_=wv_d[:, 0:3, :])
            nc.scalar.dma_start(out=wqkr_sb[:, 0:3, :], in_=wqkra_d[:, :, :])
            nc.gpsimd.dma_start(out=wqkr_sb[:, 3:6, :], in_=wqkrb_d[:, :, :])
            nc.gpsimd.dma_start(out=wo_sb, in_=wo_d[:, :, :])
            bo_ap = bo_d[:]
            nc.sync.dma_start(
                out=bias_f32,
                in_=bass.AP(tensor=bo_ap.tensor, offset=bo_ap.offset,
                            ap=[[0, 1]] + list(bo_ap.ap)),
            )
            nc.vector.memset(aoT[:, 0, 0:512], 0.0)  # prewarm-matmul source
            nc.vector.memset(vaug[:, :, :, DH], 1.0)
            warm = pp.tile([1, 2], f32)
            nc.vector.memset(warm, 0.0)
            nc.gpsimd.memset(ones_col, 1.0)
            # dummy activation: pulls the exp ACT-table load (~2.7 us) into
            # the initial DMA wait instead of the first real exp
            nc.scalar.activation(out=warm, in_=warm,
                                 func=mybir.ActivationFunctionType.Exp)

            with tc.tile_pool(name="sbC", bufs=4) as sbC, \
                 tc.tile_pool(name="sbAug", bufs=5) as sbAug, \
                 tc.tile_pool(name="sbCs", bufs=2) as sbCs, \
                 tc.tile_pool(name="psS", bufs=3, space="PSUM") as psS, \
                 tc.tile_pool(name="psF", bufs=2, space="PSUM") as psF:

                # ~4.5 us of small dummy matmuls: keeps the PE busy during
                # the input-DMA wait so HAM releases the clock gate (1.2 ->
                # 2.4 GHz) with no idle window before the first real matmul;
                # small N so the queue drains the moment real work is ready.
                wm = psF.tile([128, 512], f32, tag="fill", name="wm")
                for _ in range(40):
                    nc.tensor.matmul(wm[:, 0:128], aoT[:, 0, 0:128],
                                     aoT[:, 0, 0:128], start=True, stop=True)

                def emit_warm_fill(n):
                    # dummy-matmul filler for DMA-starved early slots
                    ps = psF.tile([128, 512], f32, tag="fill", name="wmf")
                    for _ in range(n):
                        nc.tensor.matmul(ps, aoT[:, 0, 0:128],
                                         aoT[:, 0, 0:512],
                                         start=True, stop=True)

                def emit_qk_chunk(p, qk, it):
                    # qkT[:, 2p+qk, it-half] = W_{q|k,pair p}^T @ x^T
                    ps = psF.tile([128, 512], f32, tag="fill", name="psqk")
                    for kt in range(KT):
                        if p == 0:
                            w = wqk0_sb[:, kt, 128 * qk:128 * (qk + 1)]
                        else:
                            w = wqkr_sb[:, kt, 256 * (p - 1) + 128 * qk:
                                        256 * (p - 1) + 128 * (qk + 1)]
                        nc.tensor.matmul(
                            ps, w,
                            xT_sb[:, kt, it * 512:(it + 1) * 512],
                            start=(kt == 0), stop=(kt == KT - 1),
                        )
                    nc.vector.tensor_copy(
                        out=qkT[:, 2 * p + qk, it * 512:(it + 1) * 512], in_=ps
                    )

                def emit_v_chunk(jt, et):
                    # v[jt-tile, 6 heads] = x @ W_v  (+ strided head layout)
                    ps = psF.tile([128, 384], f32, tag="fill", name="psv")
                    for kt in range(KT):
                        nc.tensor.matmul(
                            ps,
                            xT_sb[:, kt, jt * 128:(jt + 1) * 128],
                            wv_sb[:, kt, et * 384:(et + 1) * 384],
                            start=(kt == 0), stop=(kt == KT - 1),
                        )
                    nc.vector.tensor_copy(
                        out=vaug[:, jt, 6 * et:6 * (et + 1), 0:DH],
                        in_=ps.rearrange("p (h d) -> p h d", d=DH),
                    )

                p1_cmax = {}

                def emit_pass1_group(nt, et, cmax):
                    # osb_all[nt, et] = bias + sum_{ct<=cmax} aoT_ct^T @ W_out
                    # (cmax limited by which head pairs are normalized yet;
                    # pass 2 adds the rest)
                    p1_cmax[(nt, et)] = cmax
                    ps = psF.tile([128, 384], f32, tag="fill", name="psp1")
                    nc.tensor.matmul(
                        ps, ones_col, bias_bf[:, et * 384:(et + 1) * 384],
                        start=True, stop=False,
                    )
                    for ct in range(cmax + 1):
                        nc.tensor.matmul(
                            ps,
                            aoT[:, ct, nt * 128:(nt + 1) * 128],
                            wo_sb[:, ct, et * 384:(et + 1) * 384],
                            start=False, stop=(ct == cmax),
                        )
                    nc.vector.tensor_copy(out=osb_all[:, nt, et, :], in_=ps)

                def emit_pv_group(q, parity, it, st):
                    # softmax-numerator matmul group of pair q, with its own
                    # self-contained normalization chain (evac -> denominator
                    # to partition 0 -> approx-reciprocal -> broadcast ->
                    # scale). No pair-level barrier: each chain overlaps the
                    # PE work that follows its group.
                    e = st["eA"] if parity == 0 else st["eB"]
                    h = 2 * q + parity
                    ops = psF.tile([DH + 1, 512], f32, tag="fill", name="pvps")
                    for jt_ in range(NT):
                        nc.tensor.matmul(
                            ops,
                            vaug[:, jt_, h, :],
                            e[:, jt_, it * 512:(it + 1) * 512],
                            start=(jt_ == 0), stop=(jt_ == NT - 1),
                        )
                    aug = sbAug.tile([DH + 1, 512], f32, tag="aug")
                    nc.vector.tensor_copy(out=aug, in_=ops)
                    dd = sbCs.tile([1, 512], f32, tag="dd")
                    nc.vector.tensor_copy(out=dd, in_=aug[DH:DH + 1, :])
                    nc.vector.reciprocal_approx_fast(out=dd, in_=dd)
                    rbc = sbCs.tile([DH, 512], f32, tag="rsbc")
                    nc.gpsimd.partition_broadcast(rbc, dd)
                    nc.vector.tensor_mul(
                        out=aoT[parity * DH:(parity + 1) * DH, q,
                                it * 512:(it + 1) * 512],
                        in0=aug[0:DH, :],
                        in1=rbc,
                    )

                # head-start: the four q,k chunks of pair 0 accumulate
                # kt-by-kt in DMA landing order, interleaved across four
                # PSUM tiles (two fill slots + two scores slots, idle until
                # the first scores) so each x tile is consumed as it lands.
                hs = [psF.tile([128, 512], f32, tag="fill", name=f"hs{i}")
                      for i in range(2)]
                hs += [psS.tile([128, 512], f32, tag="scores", name=f"hs{i+2}")
                       for i in range(2)]
                kt_order = (0, 1, 3, 4, 2, 5)  # DMA arrival order
                for ki, kt in enumerate(kt_order):
                    for i, (qk_, it_) in enumerate(
                            ((0, 0), (0, 1), (1, 0), (1, 1))):
                        nc.tensor.matmul(
                            hs[i],
                            wqk0_sb[:, kt, 128 * qk_:128 * (qk_ + 1)],
                            xT_sb[:, kt, it_ * 512:(it_ + 1) * 512],
                            start=(ki == 0), stop=(ki == KT - 1),
                        )
                for i, (qk_, it_) in enumerate(((0, 0), (0, 1), (1, 0), (1, 1))):
                    nc.vector.tensor_copy(
                        out=qkT[:, qk_, it_ * 512:(it_ + 1) * 512], in_=hs[i]
                    )

                # per-pair filler schedules: (kind, args) lists consumed one
                # slot at a time between the scores matmuls of each jt.
                v0 = [("v", jt, 0) for jt in range(NT)]
                v1 = [("v", jt, 1) for jt in range(NT)]
                qk = lambda p: [("qk", p, q, it) for q in (0, 1) for it in (0, 1)]
                # aoT[ct] is normalized at the END of pair ct+1's loop, so
                # pass-1 filler in pair p may contract up to ct = p-2.
                # pair-0 order respects DMA arrival (wv ~17us, wqkr ~21-24us)
                # while keeping the qk chunks clear of the last slot: a qk(p)
                # item in slot 7 would be emitted after scores(p, 0).
                fillers = {
                    0: [("wm", 4, 0), ("wm", 4, 0), ("wm", 4, 0)]
                       + v0[0:5] + qk(1) + v0[5:8] + v1[:2],
                    1: qk(2) + v1[2:4],
                    2: qk(3) + v1[4:6],
                    3: qk(4) + v1[6:8],
                    4: qk(5) + [("p1", 0, 0, 2), ("p1", 0, 1, 2),
                                ("p1", 1, 0, 2)],
                    5: [("p1", nt, et, 3) for nt in range(2, 6) for et in (0, 1)]
                       + [("p1", 1, 1, 3)],
                }
                COST = {"v": 2304, "qk": 3072, "wm": 2048}

                def item_cost(item):
                    if item[0] == "p1":
                        return 384 * (item[3] + 2)
                    return COST[item[0]]

                def emit_filler(item):
                    if item[0] == "v":
                        emit_v_chunk(item[1], item[2])
                    elif item[0] == "qk":
                        emit_qk_chunk(item[1], item[2], item[3])
                    elif item[0] == "wm":
                        emit_warm_fill(item[1])
                    else:
                        emit_pass1_group(item[1], item[2], item[3])

                def emit_scores(p, jt):
                    # head 2p at array rows 0:64 and head 2p+1 at rows 64:128
                    # run as concurrent row-tiled pairs; the 3-deep psS
                    # rotation frees destination banks >=2 exp periods early
                    # so these never stall and the exp stream stays gapless.
                    sA = psS.tile([128, N], f32, tag="scores", name="sA")
                    sB = psS.tile([128, N], f32, tag="scores", name="sB")
                    for it in range(2):
                        nc.tensor.matmul(
                            sA[:, it * 512:(it + 1) * 512],
                            qkT[0:DH, 2 * p + 1, jt * 128:(jt + 1) * 128],
                            qkT[0:DH, 2 * p, it * 512:(it + 1) * 512],
                            start=True, stop=True,
                        )
                    for it in range(2):
                        nc.tensor.matmul(
                            sB[:, it * 512:(it + 1) * 512],
                            qkT[DH:128, 2 * p + 1, jt * 128:(jt + 1) * 128],
                            qkT[DH:128, 2 * p, it * 512:(it + 1) * 512],
                            start=True, stop=True,
                        )
                    return sA, sB

                # flat slot schedule over all (pair, jt) exp slots; the
                # scores pipeline crosses pair boundaries so ScalarE never
                # waits at a transition. A slot holds ~4200 spare PE cycles
                # (~150 when the previous pair's PV group occupies it).
                # last PV group at slot 6 (not 7): it is the final reader of
                # the two-pairs-back exp tiles, and the next pair's first exp
                # needs that buffer slot — slot 7 placement stalls exp(p, 0).
                pv_slots = {1: (0, 0), 3: (0, 1), 5: (1, 0), 6: (1, 1)}
                plan = []
                for p in range(NPAIR):
                    # proportional-by-cost placement over the cumulative
                    # spare capacity of the pair's 8 slots (a PV-occupied
                    # slot has almost none); items keep list order, so
                    # deadline ordering is by construction.
                    cap = [4270 - (4096 if (p > 0 and jt in pv_slots) else 0)
                           for jt in range(NT)]
                    total_cost = sum(item_cost(i) for i in fillers[p])
                    total_cap = sum(cap)
                    scale = min(1.0, total_cap / max(1, total_cost))
                    slot_items = [[] for _ in range(NT)]
                    cum = 0.0
                    cumcap = 0.0
                    sj = 0
                    for item in fillers[p]:
                        c = item_cost(item) * scale
                        while sj < NT - 1 and cumcap + cap[sj] < cum + c * 0.5:
                            cumcap += cap[sj]
                            sj += 1
                        slot_items[sj].append(item)
                        cum += c
                    for jt in range(NT):
                        plan.append((p, jt, slot_items[jt]))

                states = {}

                def pair_state(p):
                    if p not in states:
                        states[p] = {
                            "eA": sbC.tile([128, NT, N], bf16, tag="expT",
                                           name="eA"),
                            "eB": sbC.tile([128, NT, N], bf16, tag="expT",
                                           name="eB"),
                        }
                    return states[p]

                sAB = emit_scores(0, 0)
                for si, (p, jt, items) in enumerate(plan):
                    st = pair_state(p)
                    nc.scalar.activation(
                        out=st["eA"][:, jt, :], in_=sAB[0],
                        func=mybir.ActivationFunctionType.Exp, scale=SCALE,
                    )
                    nc.scalar.activation(
                        out=st["eB"][:, jt, :], in_=sAB[1],
                        func=mybir.ActivationFunctionType.Exp, scale=SCALE,
                    )
                    # keep the scores one slot ahead of the exp stream,
                    # across pair boundaries
                    if si + 1 < len(plan):
                        np_, njt, _ = plan[si + 1]
                        sAB = emit_scores(np_, njt)
                    if p == 1 and jt == 0:
                        nc.vector.tensor_copy(out=bias_bf, in_=bias_f32)
                    # previous pair's PV group, then filler, as PE cover
                    if p > 0 and jt in pv_slots:
                        parity, it = pv_slots[jt]
                        emit_pv_group(p - 1, parity, it, pair_state(p - 1))
                    for item in items:
                        emit_filler(item)

                # drain: all of the last pair's PV groups first so their
                # normalize chains stream on DVE without p1-evac head-of-line
                # blocking; the p1 groups follow as PE cover for the chains.
                for parity in (0, 1):
                    for it in range(2):
                        emit_pv_group(NPAIR - 1, parity, it,
                                      pair_state(NPAIR - 1))
                for nt in (6, 7):
                    for et in (0, 1):
                        emit_pass1_group(nt, et, 4)

            # ---- stage D pass 2: per (nt, et) add the remaining ct products
            # (the head pairs that weren't normalized when pass 1 ran) to the
            # pass-1 partials and store one full 3 KB row block per nt,
            # alternating store queues.
            with tc.tile_pool(name="sbDo", bufs=4) as sbDo, \
                 tc.tile_pool(name="psD2", bufs=4, space="PSUM") as psD2:
                # nt 6,7 (single ct5 matmul) first: they absorb the one
                # unavoidable wait on the last pair's normalization; the
                # deeper ct3..5 groups for nt 0,1 then run stall-free.
                for nt in (6, 7, 0, 1, 2, 3, 4, 5):
                    osb = sbDo.tile([128, D], f32, tag="osb")
                    # both et halves share one bank-padded PSUM tile (each
                    # matmul group stays within its own 2 KB bank) so a
                    # single strided DVE add finishes the whole row block
                    ps = psD2.tile([128, 1024], f32, tag="ops2", bufs=2)
                    for et in range(2):
                        cts = list(range(p1_cmax[(nt, et)] + 1, KT))
                        for i, ct in enumerate(cts):
                            nc.tensor.matmul(
                                ps[:, et * 512:et * 512 + 384],
                                aoT[:, ct, nt * 128:(nt + 1) * 128],
                                wo_sb[:, ct, et * 384:(et + 1) * 384],
                                start=(i == 0), stop=(i == len(cts) - 1),
                            )
                    nc.vector.tensor_add(
                        out=osb.rearrange("p (e w) -> p e w", w=384),
                        in0=ps.rearrange("p (e w) -> p e w", w=512)[:, :, 0:384],
                        in1=osb_all[:, nt, :, :],
                    )
                    eng = (nc.sync, nc.gpsimd, nc.scalar)[nt % 3]
                    eng.dma_start(
                        out=out_d[nt * 128:(nt + 1) * 128, :], in_=osb
                    )
    nc.compile()
    return nc


_CACHE = {}


def _get_nc():
    if "nc" not in _CACHE:
        _CACHE["nc"] = build_bass()
    return _CACHE["nc"]


def _pack_kt(a):
    # [768, C] -> [128, 6, C] with row kt*128+p on partition p, block kt
    C = a.shape[1]
    return np.ascontiguousarray(a.reshape(KT, 128, C).transpose(1, 0, 2))


def _make_in_maps(x, w_qkv, w_out, b_out):
    bf = ml_dtypes.bfloat16
    x = np.asarray(x, dtype=np.float32)
    wq = np.asarray(w_qkv, dtype=np.float32)
    # pair-packed q|k columns: [q_p0 | k_p0 | q_p1 | k_p1 | ...]
    qk = np.empty((D, 2 * D), dtype=np.float32)
    for p in range(NPAIR):
        qk[:, 256 * p:256 * p + 128] = wq[:, 128 * p:128 * (p + 1)]
        qk[:, 256 * p + 128:256 * p + 256] = wq[:, D + 128 * p:D + 128 * (p + 1)]
    qk_pack = _pack_kt(qk).astype(bf)
    wqk0 = np.ascontiguousarray(qk_pack[:, :, 0:256])
    wqkr = qk_pack[:, :, 256:]
    wqkra = np.ascontiguousarray(wqkr[:, 0:3, :])
    wqkrb = np.ascontiguousarray(wqkr[:, 3:6, :])
    wv = np.ascontiguousarray(_pack_kt(wq[:, 2 * D:]).astype(bf))
    wo = np.ascontiguousarray(_pack_kt(np.asarray(w_out, dtype=np.float32)).astype(bf))
    bo = np.ascontiguousarray(np.asarray(b_out, dtype=np.float32))
    in_maps = []
    for b in range(B):
        xT = _pack_kt(np.ascontiguousarray(x[b].T)).astype(bf)
        m = {"wqk0": wqk0, "wqkra": wqkra, "wqkrb": wqkrb,
             "wv": wv, "wo": wo, "bo": bo}
        for kt in range(KT):
            m[f"x{kt}"] = np.ascontiguousarray(xT[:, kt, :])
        in_maps.append(m)
    return in_maps


def kernel(x, w_qkv, w_out, b_out):
    nc = _get_nc()
    in_maps = _make_in_maps(x, w_qkv, w_out, b_out)
    res = run_bass_kernel_spmd(nc, in_maps, list(range(B)))
    return np.stack([res.results[b]["out"] for b in range(B)]).astype(np.float32)


# ---------------------------------------------------------------------------
# profiling helper (used by test.py only; safe no-op fallback if the axon
# NTFF hook infrastructure is unavailable)
def _install_profhook():
    import sys
    import types

    if "antenv.axon_hooks" in sys.modules:
        return True
    try:
        import antenv
        from trn_agent_boot.trn_boot import _ntff_profile_via_ctypes

        hook = _ntff_profile_via_ctypes("/opt/axon/libaxon_pjrt.so")
        mod = types.ModuleType("antenv.axon_hooks")
        mod._hook = hook
        mod.get_axon_ntff_profile_hook = lambda: mod._hook

        def _set(h):
            mod._hook = h

        mod.set_axon_ntff_profile_hook = _set
        sys.modules["antenv.axon_hooks"] = mod
        antenv.axon_hooks = mod

        import concourse.bass_utils as bu

        bu.upload_artifacts = lambda tmpdir: f"local:{tmpdir}"
        return True
    except Exception as e:  # pragma: no cover
        print(f"profhook install failed: {e}")
        return False


def run_traced(x, w_qkv, w_out, b_out, tmpdir=None):
    """Run with NTFF profiling; returns (out, exec_time_ns, results_obj)."""
    traced = _install_profhook()
    nc = _get_nc()
    in_maps = _make_in_maps(x, w_qkv, w_out, b_out)
    res = run_bass_kernel_spmd(
        nc, in_maps, list(range(B)), trace=traced, tmpdir=tmpdir
    )
    out = np.stack([res.results[b]["out"] for b in range(B)]).astype(np.float32)
    return out, res.exec_time_ns, res
